# revision 48
# baseline (speedup 1.0000x reference)
"""ConvSP (SPH message-passing conv) Trainium2 kernel.

Problem (per full input): B=2 batches, N=8192 particles, M=8192 queries,
K=32 neighbors, C=16 in channels, O=16 out channels, 27 kernel cells.

    out[b,m,o] = bias[o] + sum_{e,k,c} kv(b,m,e,k) * data[b, nbr[b,m,k], c] * W[o,c,e]
    kv = relu(1 - sqrt(|qloc + off_e - loc_nbr|^2 + 1e-12)/R)^3

Sharding: 8 cores = 2 batches x 4 query-quarters (2048 queries each), SPMD.

Host prep (numpy, off HW-time): packed particle records rec[n] =
[lx,ly,lz f32 | data bf16 x16] in 256B-strided rows; qT3 query broadcast
[(t,k), (j,d)]; wrap-format int16 gather indices; block-diag weights.

Per-core dataflow (chunk = 4 queries m=4j+t; partition (t,k) = t*32+k):
  - neighbor records fetched with a raw dma_gather (elem 64B, row stride
    256B): the natural flat neighbor stream s = m*32+k lands at
    out[(t,k), j, :] (p = s%128, chunk = s//128).
  - distances on DVE via the separable cell-offset factorization
    d2 = |delta|^2 + sum_axis(2*off*delta + off^2); kv = relu(1-d/R)^3
    via ACT sqrt + ACT relu-affine + DVE squares.
  - data c-fields copied into a block-diagonal "slotted" tile (zeros
    elsewhere, memset once); acc[(t,c), e] per chunk via one PE matmul
    (lhsT = slotted data [128, 64], rhs = dense kv [128, 27]).
  - out[(t,o), j] via 27 accumulated PE matmuls against host-prepped
    block-diagonal W; bias fused in the ACT PSUM->SBUF copy; PE transpose
    to [j, (t,o)] for a contiguous store.
"""
import os
import sys
import numpy as np
from contextlib import ExitStack

sys.path.insert(0, "/opt/trn_rl_repo")

import concourse.bass as bass
import concourse.bacc as bacc
import concourse.mybir as mybir
import concourse.tile as tile
import concourse.ap_utils as ap_utils
from concourse.masks import make_identity
from concourse.bass_utils import run_bass_kernel_spmd

F32 = mybir.dt.float32
BF16 = mybir.dt.bfloat16
I32 = mybir.dt.int32
I16 = mybir.dt.int16
AF = mybir.ActivationFunctionType

P = 128          # partitions
NQ = 2048        # queries per core
N = 8192         # particles per batch
K = 32           # neighbors
C = 16           # in channels
O = 16           # out channels
D = 3
E = 27           # cells
RECF = 16        # gathered record fields (f32): lx,ly,lz + 16 bf16 -> 64B
RECS = 64        # record row stride fields (f32): 256B rows (DGE stride quantum)
T = 4            # queries per chunk
J = NQ // T      # chunks per core = 512
JS = 64          # chunks per gather block == compute subtile
NSUB = J // JS   # 8
RADIUS = 0.1
DIL = 0.05
STAGE = int(os.environ.get("CONVSP_STAGE", "5"))
EJ = os.environ.get("CONVSP_EJ", "1") == "1"


def raw_dma_gather(eng, out_ap, in_ap, idxs_ap, num_idxs, elem_size, elem_step,
                   queue_num=0, single_packet=False):
    """dma_gather (transpose=False, DRAM source) without the %256
    elem-size assert — that restriction is only needed for transpose
    mode's 16-bit partition shuffle. elem_step (row stride) must still
    be a multiple of 256B, which is the DGE stride quantum."""
    eng._assert_queue_num(queue_num)
    assert idxs_ap.dtype == mybir.dt.int16
    assert in_ap.space == bass.MemorySpace.DRAM
    assert idxs_ap.space == bass.MemorySpace.SBUF
    assert out_ap.space == bass.MemorySpace.SBUF
    assert in_ap.dtype == out_ap.dtype
    assert ap_utils.ap_is_contiguous(in_ap.ap[1:])
    assert ap_utils.ap_is_contiguous(out_ap.ap[1:])
    assert ap_utils.ap_is_contiguous(idxs_ap.ap[1:])
    assert in_ap.ap[-1][1] == out_ap.ap[-1][1] == elem_size
    assert out_ap.ap[0][1] * out_ap.ap[1][1] == num_idxs  # num_idxs % 128 == 0
    assert in_ap.ap[0][0] == elem_step
    stride_bytes = elem_step * mybir.dt.size(in_ap.dtype)
    assert stride_bytes % 256 == 0
    stride_bytes_256 = stride_bytes // 256
    assert stride_bytes_256 < 256

    _in_ap = eng.lower_ap_dma(in_ap, for_custom_bir_dma=True)
    inst = eng.add_instruction(
        mybir.InstDMAGatherAnt(
            name=eng.bass.get_next_instruction_name(),
            ins=[
                *_in_ap,
                eng.lower_ap(idxs_ap),
                eng.lower_val_access(eng.to_reg(num_idxs)),
            ],
            outs=[eng.lower_ap(out_ap)],
            transpose=False,
            num_idxs=num_idxs,
            elem_size=elem_size,
            stride_bytes_256=stride_bytes_256,
            gen_mode=0,
            single_packet=single_packet,
            queue_num=queue_num,
            sbuf_tokens_per_rank=0,
            sbuf_free_dim_per_rank=0,
            sbuf_free_dim_pad_per_rank=0,
            sbuf_byte_offset=0,
        )
    )
    return inst


def build_program():
    nc = bacc.Bacc("TRN2", target_bir_lowering=False, debug=False,
                   num_devices=8, num_swdge_queues=4)

    rec_d = nc.declare_dram_parameter("rec", [N, RECS], F32, isOutput=False)
    qT3_d = nc.declare_dram_parameter("qT3", [P, J * D], F32, isOutput=False)
    w16_d = nc.declare_dram_parameter("w16", [16, J * 8], I16, isOutput=False)
    wbd_d = nc.declare_dram_parameter("wbd", [64, E * 64], BF16, isOutput=False)
    bias4_d = nc.declare_dram_parameter("bias4", [64], F32, isOutput=False)
    out_d = nc.declare_dram_parameter("out", [NQ, O], F32, isOutput=True)

    with tile.TileContext(nc) as tc:
        with ExitStack() as ctx:
            _build(ctx, tc, rec_d, qT3_d, w16_d, wbd_d, bias4_d, out_d)
    nc.finalize()
    return nc


def _build(ctx, tc, rec_d, qT3_d, w16_d, wbd_d, bias4_d, out_d):
    nc = tc.nc

    consts = ctx.enter_context(tc.tile_pool(name="consts", bufs=1))
    gpool = ctx.enter_context(tc.tile_pool(name="gpool", bufs=1))
    dve = ctx.enter_context(tc.tile_pool(name="dve", bufs=3))
    kvp = ctx.enter_context(tc.tile_pool(name="kv", bufs=3))
    accs = ctx.enter_context(tc.tile_pool(name="accs", bufs=1))
    outs = ctx.enter_context(tc.tile_pool(name="outs", bufs=int(os.environ.get("CONVSP_OUTS", "3"))))
    accps = ctx.enter_context(tc.tile_pool(name="accps", bufs=int(os.environ.get("CONVSP_APS", "4")), space="PSUM"))
    outps = ctx.enter_context(tc.tile_pool(name="outps", bufs=2, space="PSUM"))
    trps = ctx.enter_context(tc.tile_pool(name="trps", bufs=2, space="PSUM"))

    # ---------------- constants ----------------
    oxc = consts.tile([P, D], F32)      # 2*off(e)
    ox2c = consts.tile([P, D], F32)     # off(e)^2
    for i in range(D):
        off = (i - 1) * DIL
        nc.vector.memset(oxc[:, i:i + 1], 2.0 * off)
        nc.vector.memset(ox2c[:, i:i + 1], off * off)
    epsb = consts.tile([P, 1], F32)
    nc.vector.memset(epsb[:], 1e-12)
    oneb = consts.tile([P, 1], F32)
    nc.vector.memset(oneb[:], 1.0)
    ident = consts.tile([64, 64], F32)
    make_identity(nc, ident[:])
    # w16/qT3 loaded in per-subtile chunks so gather 0 / subtile 0 start
    # as soon as their slice lands; wbd/bias only gate the first final group
    w16 = consts.tile([P, J * 8], I16)  # [128, 4096]
    WC = J * 8 // NSUB
    QC = J * D // NSUB
    w16v = w16[:].rearrange("p (s c) -> p s c", s=NSUB)
    qT3 = consts.tile([P, J * D], F32)
    qT3v = qT3[:].rearrange("p (s c) -> p s c", s=NSUB)

    def w16_src(s):
        return w16_d[:, s * WC:(s + 1) * WC].partition_broadcast(8)

    def load_w16_chunk(s):
        nc.sync.dma_start(w16v[:, s], w16_src(s))

    def load_qT3_chunk(s):
        nc.scalar.dma_start(qT3v[:, s], qT3_d[:, s * QC:(s + 1) * QC])

    load_w16_chunk(0)
    load_w16_chunk(1)
    wbd = consts.tile([64, E * 64], BF16)
    bias4 = consts.tile([64, 1], F32)

    # ---------------- gather + slotted buffers ----------------
    NGB = 4
    NSB = 3
    gbufs = [gpool.tile([P, JS * RECF], F32, tag=f"g{i}", name=f"g{i}")
             for i in range(NGB)]
    sbufs = [gpool.tile([P, JS * 64], BF16, tag=f"s{i}", name=f"s{i}")
             for i in range(NSB)]

    def gather_block(s):
        sm = s % NSUB
        g = gbufs[s % NGB]
        gvr = g[:].rearrange("p (j f) -> p j f", f=RECF)
        NSPL = int(os.environ.get("CONVSP_NSPL", "2"))
        cpq = JS // NSPL
        icols = JS * 8 // NSPL
        for q in range(NSPL):
            raw_dma_gather(
                nc.gpsimd,
                out_ap=gvr[:, q * cpq:(q + 1) * cpq, :],
                in_ap=rec_d[:, 0:RECF],
                idxs_ap=w16[:, sm * (JS * 8) + q * icols:
                            sm * (JS * 8) + (q + 1) * icols],
                num_idxs=cpq * P,
                elem_size=RECF,
                elem_step=RECS,
                queue_num=q,
            )

    def sub_pre(s):
        sm = s % NSUB
        g = gbufs[s % NGB]
        sbd = sbufs[s % NSB]
        gv = g[:].rearrange("p (j f) -> p j f", f=RECF)

        # --- block-diag data: t=0,2 as raw byte moves on DMA rings now;
        # t=1,3 as DVE 4x bf16 copies issued AFTER the kv chain so they
        # don't head-of-line-block d3 in the in-order DVE queue
        gb = g[:].bitcast(BF16).rearrange("p (j f) -> p j f", f=RECF * 2)
        sv = sbd[:].rearrange("p (j s c) -> p j s c", s=T, c=C)

        def slot(t):
            return (sv[t * K:(t + 1) * K, :, t, :],
                    gb[t * K:(t + 1) * K, :, 2 * D:2 * D + C])

        for t, eng in ((0, nc.sync), (2, nc.scalar)):
            dst, src = slot(t)
            eng.dma_start(dst, src)

        if EJ:
            # --- (e,j)-major per-axis-squares path: every op's inner dim is
            # the packed j run, so the broadcast adds hit DVE 2x bf16 mode;
            # summands (d_a+v)^2 are individually small => bf16-safe
            qs = qT3v[:, sm].rearrange("p (d j) -> p d j", j=JS)
            d3b = dve.tile([P, D * JS], BF16, tag="d3b")
            d3bv = d3b[:].rearrange("p (d j) -> p d j", j=JS)
            for d in range(D):
                nc.vector.tensor_sub(d3bv[:, d, :], qs[:, d, :], gv[:, :, d])
            # t[a, v, j] = d_a + off_v  (3 fused-scalar adds, 4x mode)
            tav = dve.tile([P, 9 * JS], BF16, tag="tav")
            tavv = tav[:].rearrange("p (a v j) -> p a v j", a=D, v=D)
            for vb in range(D):
                nc.vector.tensor_scalar_add(tavv[:, :, vb, :], d3bv,
                                            (vb - 1) * DIL)
            # w = t^2 in one 2x mul
            wav = dve.tile([P, 9 * JS], BF16, tag="wav")
            nc.vector.tensor_mul(wav[:], tav[:], tav[:])
            wv = wav[:].rearrange("p (a v j) -> p a v j", a=D, v=D)
            wx, wy, wz = wv[:, 0], wv[:, 1], wv[:, 2]
            # u2[vx, vy, j] ; d2[vx, vy, vz, j] — 2x adds, packed inner j
            u2 = dve.tile([P, 9 * JS], BF16, tag="u2e")
            u2v = u2[:].rearrange("p (x y j) -> p x y j", x=D, y=D)
            nc.vector.tensor_add(
                u2v,
                wx.unsqueeze(2).broadcast_to((P, D, D, JS)),
                wy.unsqueeze(1).broadcast_to((P, D, D, JS)))
            d2 = kvp.tile([P, JS * E], BF16, tag="d2")
            d2v = d2[:].rearrange("p (u z j) -> p u z j", u=9, z=D)
            nc.vector.tensor_add(
                d2v,
                u2[:].rearrange("p (u j) -> p u j", j=JS).unsqueeze(2)
                     .broadcast_to((P, 9, D, JS)),
                wz.unsqueeze(1).broadcast_to((P, 9, D, JS)))
            d2f = d2
            nc.scalar.activation(d2f[:], d2f[:], AF.Sqrt, bias=epsb[:])
            y = kvp.tile([P, JS * E], BF16, tag="y")
            nc.scalar.activation(y[:], d2f[:], AF.Relu, bias=oneb[:],
                                 scale=-1.0 / RADIUS)
            return dict(s=s, sbd=sbd, slot=slot, y=y)

        # --- delta = q - l
        d3 = dve.tile([P, JS * D], F32, tag="d3")
        q_v = qT3[:].rearrange("p (j d) -> p j d", d=D)
        nc.vector.tensor_sub(
            d3[:].rearrange("p (j d) -> p j d", d=D),
            q_v[:, sm * JS:(sm + 1) * JS, :],
            gv[:, :, 0:D])

        d3v = d3[:].rearrange("p (j d) -> p j d", d=D)
        dx, dy, dz = d3v[:, :, 0], d3v[:, :, 1], d3v[:, :, 2]

        # --- s2 = dx^2+dy^2+dz^2
        s2 = dve.tile([P, JS], F32, tag="s2")
        tmp = dve.tile([P, JS], F32, tag="tmp")
        nc.vector.tensor_mul(s2[:], dx, dx)
        nc.vector.tensor_mul(tmp[:], dy, dy)
        nc.vector.tensor_add(s2[:], s2[:], tmp[:])
        nc.vector.tensor_mul(tmp[:], dz, dz)
        nc.vector.tensor_add(s2[:], s2[:], tmp[:])

        # --- per-axis terms p*[j,e] = 2*off_e*d + off_e^2 (+ s2 on x),
        # one fused scalar op per cell offset (off_e is a literal)
        def axis_term(dcomp, add_s2, tg):
            pt = dve.tile([P, JS * D], F32, tag=tg)
            ptv = pt[:].rearrange("p (j e) -> p j e", e=D)
            for e in range(D):
                off = (e - 1) * DIL
                if add_s2:
                    nc.vector.affine_then_add(ptv[:, :, e], dcomp, s2[:],
                                              scale=2.0 * off, bias=off * off)
                else:
                    nc.vector.tensor_scalar(ptv[:, :, e], dcomp,
                                            2.0 * off, off * off,
                                            mybir.AluOpType.mult,
                                            mybir.AluOpType.add)
            return pt

        pxe = axis_term(dx, True, "pxe")
        pye = axis_term(dy, False, "pye")
        pze = axis_term(dz, False, "pze")

        # --- u2[j,ex,ey] = pxe+pye ; d2[j,ex,ey,ez] = u2+pze
        u2 = dve.tile([P, JS * 9], F32, tag="u2")
        u2v = u2[:].rearrange("p (j a b) -> p j a b", a=D, b=D)
        nc.vector.tensor_add(
            u2v,
            pxe[:].rearrange("p (j a) -> p j a", a=D).unsqueeze(3)
                  .broadcast_to((P, JS, D, D)),
            pye[:].rearrange("p (j b) -> p j b", b=D).unsqueeze(2)
                  .broadcast_to((P, JS, D, D)))
        d2 = kvp.tile([P, JS * E], F32, tag="d2")
        d2v = d2[:].rearrange("p (j a b) -> p j a b", a=9, b=D)
        u2b = u2[:].rearrange("p (j a) -> p j a", a=9).unsqueeze(3) \
                   .broadcast_to((P, JS, 9, D))
        pzb = pze[:].rearrange("p (j b) -> p j b", b=D).unsqueeze(2) \
                    .broadcast_to((P, JS, 9, D))
        H = int(os.environ.get("CONVSP_D2POOL", "0"))
        if H:
            # split the biggest DVE op: first H j-columns on Pool
            nc.gpsimd.tensor_add(d2v[:, 0:H], u2b[:, 0:H], pzb[:, 0:H])
            nc.vector.tensor_add(d2v[:, H:], u2b[:, H:], pzb[:, H:])
        else:
            nc.vector.tensor_add(d2v, u2b, pzb)

        # --- kv = relu(1 - sqrt(d2+eps)/R)^3; y bf16 so the two cube muls
        # run in DVE 2x packed-16-bit mode
        nc.scalar.activation(d2[:], d2[:], AF.Sqrt, bias=epsb[:])
        y = kvp.tile([P, JS * E], BF16, tag="y")
        nc.scalar.activation(y[:], d2[:], AF.Relu, bias=oneb[:],
                             scale=-1.0 / RADIUS)
        return dict(s=s, sbd=sbd, slot=slot, y=y)

    def sub_post(st):
        s, sbd, slot, y = st["s"], st["sbd"], st["slot"], st["y"]
        sq = kvp.tile([P, JS * E], BF16, tag="sq")
        nc.vector.tensor_mul(sq[:], y[:], y[:])
        kv = kvp.tile([P, JS * E], BF16, tag="kvt")
        nc.vector.tensor_mul(kv[:], sq[:], y[:])

        for t in (1, 3):
            dst, src = slot(t)
            if os.environ.get("CONVSP_SLOTDMA", "0") == "1":
                (nc.sync if t == 1 else nc.scalar).dma_start(dst, src)
            else:
                nc.vector.tensor_copy(dst, src)

        # --- acc[(t,c), e] per chunk on PE (bf16: single-pass PE matmul)
        if EJ:
            kvv = kv[:].rearrange("p (e j) -> p j e", j=JS)
        else:
            kvv = kv[:].rearrange("p (j e) -> p j e", e=E)
        acc_sb = acc2[s % 2]
        for grp in range(JS // 16):
            ap_ps = accps.tile([64, 16 * E], F32, tag="accps")
            for jl in range(16):
                jj = grp * 16 + jl
                nc.tensor.matmul(ap_ps[:, jl * E:(jl + 1) * E],
                                 sbd[:, jj * 64:(jj + 1) * 64],
                                 kvv[:, jj, :],
                                 start=True, stop=True)
            nc.scalar.activation(acc_sb[:, grp * 16 * E:(grp + 1) * 16 * E],
                                 ap_ps[:], AF.Copy)

        final_group(s)

    def final_group(fg):
        # --- out[(t,o), j] = sum_e Wbd_e @ acc_e, one subtile (64 cols)
        JT = JS
        op = outps.tile([64, JT], F32, tag="outps")
        accv = acc2[fg % 2].rearrange("p (jt e) -> p jt e", e=E)
        for e in range(E):
            nc.tensor.matmul(op[:], wbd[:, e * 64:(e + 1) * 64],
                             accv[:, :, e],
                             start=(e == 0), stop=(e == E - 1))
        osb = outs.tile([64, JT], F32, tag="osb")
        nc.scalar.activation(osb[:], op[:], AF.Identity, bias=bias4[:])

        # --- transpose to [j, (t,o)] and store contiguously
        out_v = out_d[:].rearrange("(s j t) o -> s j (t o)", s=NSUB, t=T)
        for q in range(JT // 64):
            trp = trps.tile([64, 64], F32, tag="trp")
            nc.tensor.transpose(trp[:], osb[:, q * 64:(q + 1) * 64], ident[:])
            trs = outs.tile([64, 64], F32, tag="trs")
            nc.scalar.activation(trs[:], trp[:], AF.Copy)
            nc.sync.dma_start(out_v[fg % NSUB], trs[:])

    accbig = accs.tile([64, 2 * JS * E], BF16, tag="accbig", name="accbig")
    acc2 = [accbig[:, i * JS * E:(i + 1) * JS * E] for i in range(2)]

    # ---------------- pipeline ----------------
    # slotted-buffer zeroing on DVE: it is idle during the first gather.
    # sub_post(g-1) is issued after sub_pre(g) so the DVE ops that depend
    # on ACT output never head-of-line-block the next subtile's DVE work.
    reps = int(os.environ.get("CONVSP_REPS", "1"))
    total = NSUB * reps
    for b in range(NSB):
        nc.vector.memset(sbufs[b][:], 0.0)
    for gg in range(int(os.environ.get("CONVSP_GA", "2"))):
        if 2 <= gg < NSUB:
            load_w16_chunk(gg)
        gather_block(gg)
    load_qT3_chunk(0)
    load_qT3_chunk(1)
    nc.scalar.dma_start(wbd[:], wbd_d[:])
    nc.sync.dma_start(bias4[:], bias4_d[:].rearrange("(p o) -> p o", o=1))
    pending = []
    for g in range(total):
        GA = int(os.environ.get("CONVSP_GA", "2"))
        if g + GA < total:
            if 2 <= g + GA < NSUB:
                load_w16_chunk(g + GA)
            gather_block(g + GA)
        if 2 <= g + 1 < NSUB:
            load_qT3_chunk(g + 1)
        st = sub_pre(g)
        pending.append(st)
        if len(pending) > int(os.environ.get("CONVSP_PD", "1")):
            sub_post(pending.pop(0))
    while pending:
        sub_post(pending.pop(0))


_PROGRAM = None


def _get_program():
    global _PROGRAM
    if _PROGRAM is None:
        _PROGRAM = build_program()
    return _PROGRAM


def kernel(qlocs, locs, data, neighbors, weight, bias):
    import ml_dtypes

    B, M = qlocs.shape[0], qlocs.shape[1]
    assert (B, M) == (2, 8192)
    ncores = 8

    # host-side constant/layout prep: block-diagonal weights + replicated bias
    wbd = np.zeros((E, 64, 64), np.float32)
    w = np.asarray(weight, np.float32)           # [O, C, E]
    for t in range(T):
        # wbd[e, (t,c), (t,o)] = w[o, c, e]
        wbd[:, t * C:(t + 1) * C, t * O:(t + 1) * O] = w.transpose(2, 1, 0)
    wbd = np.ascontiguousarray(
        wbd.transpose(1, 0, 2).reshape(64, E * 64)).astype(ml_dtypes.bfloat16)
    bias4 = np.tile(np.asarray(bias, np.float32), T)

    # packed particle records: [lx,ly,lz f32 | data bf16 x16 | pad] per 256B row
    recs = []
    for b in range(B):
        rec = np.zeros((N, RECS), np.float32)
        rec[:, 0:D] = locs[b]
        rec.view(np.uint16)[:, 2 * D:2 * D + C] = (
            np.asarray(data[b], np.float32).astype(ml_dtypes.bfloat16)
            .view(np.uint16))
        recs.append(rec)

    in_maps = []
    for cid in range(ncores):
        b, qq = cid // 4, cid % 4
        sl = slice(qq * NQ, (qq + 1) * NQ)
        q = np.asarray(qlocs[b, sl], np.float32)          # [NQ, D]
        # qT3[(t*32+k), (j,d)] = q[4j+t, d]
        if EJ:
            # chunk-local (d, j)-major: qT3[p, (s, d, j64)] = q[4(s*64+j)+t, d]
            qsd = q.reshape(NSUB, JS, T, D).transpose(2, 0, 3, 1)  # [T,s,d,j]
            qT3 = np.ascontiguousarray(
                np.broadcast_to(qsd[:, None, :, :, :],
                                (T, K, NSUB, D, JS)).reshape(P, J * D))
        else:
            qT3 = np.ascontiguousarray(
                np.broadcast_to(
                    q.reshape(J, T, D).transpose(1, 0, 2)[:, None, :, :],
                    (T, K, J, D)).reshape(P, J * D))
        # wrap-format gather indices: flat stream i = m*32+k; w16[r, c] = flat[c*16+r]
        flat = np.asarray(neighbors[b, sl], np.int64).reshape(-1)
        w16 = np.ascontiguousarray(
            flat.reshape(J * 8, 16).T.astype(np.int16))
        in_maps.append({
            "rec": recs[b],
            "qT3": qT3,
            "w16": w16,
            "wbd": wbd,
            "bias4": bias4,
        })

    nc = _get_program()
    res = run_bass_kernel_spmd(nc, in_maps, list(range(ncores)),
                               trace=bool(int(os.environ.get("CONVSP_TRACE", "0"))))
    out = np.zeros((B, M, O), np.float32)
    for cid in range(ncores):
        b, qq = cid // 4, cid % 4
        out[b, qq * NQ:(qq + 1) * NQ] = res.results[cid]["out"]
    kernel.last_results = res
    return out


# revision 56
# speedup vs baseline: 1.0957x; 1.0957x over previous
"""ConvSP (SPH message-passing conv) Trainium2 kernel.

Problem (per full input): B=2 batches, N=8192 particles, M=8192 queries,
K=32 neighbors, C=16 in channels, O=16 out channels, 27 kernel cells.

    out[b,m,o] = bias[o] + sum_{e,k,c} kv(b,m,e,k) * data[b, nbr[b,m,k], c] * W[o,c,e]
    kv = relu(1 - sqrt(|qloc + off_e - loc_nbr|^2 + 1e-12)/R)^3

Sharding: 8 cores = 2 batches x 4 query-quarters (2048 queries each), SPMD.

Host prep (numpy, off HW-time): packed particle records rec[n] =
[lx,ly,lz f32 | data bf16 x16] in 256B-strided rows; qT3 query broadcast
[(t,k), (j,d)]; wrap-format int16 gather indices; block-diag weights.

Per-core dataflow (chunk = 4 queries m=4j+t; partition (t,k) = t*32+k):
  - neighbor records fetched with a raw dma_gather (elem 64B, row stride
    256B): the natural flat neighbor stream s = m*32+k lands at
    out[(t,k), j, :] (p = s%128, chunk = s//128).
  - distances on DVE via the separable cell-offset factorization
    d2 = |delta|^2 + sum_axis(2*off*delta + off^2); kv = relu(1-d/R)^3
    via ACT sqrt + ACT relu-affine + DVE squares.
  - data c-fields copied into a block-diagonal "slotted" tile (zeros
    elsewhere, memset once); acc[(t,c), e] per chunk via one PE matmul
    (lhsT = slotted data [128, 64], rhs = dense kv [128, 27]).
  - out[(t,o), j] via 27 accumulated PE matmuls against host-prepped
    block-diagonal W; bias fused in the ACT PSUM->SBUF copy; PE transpose
    to [j, (t,o)] for a contiguous store.
"""
import os
import sys
import numpy as np
from contextlib import ExitStack

sys.path.insert(0, "/opt/trn_rl_repo")

import concourse.bass as bass
import concourse.bacc as bacc
import concourse.mybir as mybir
import concourse.tile as tile
import concourse.ap_utils as ap_utils
from concourse.masks import make_identity
from concourse.bass_utils import run_bass_kernel_spmd

F32 = mybir.dt.float32
BF16 = mybir.dt.bfloat16
I32 = mybir.dt.int32
I16 = mybir.dt.int16
AF = mybir.ActivationFunctionType

P = 128          # partitions
NQ = 2048        # queries per core
N = 8192         # particles per batch
K = 32           # neighbors
C = 16           # in channels
O = 16           # out channels
D = 3
E = 27           # cells
RECF = 16        # gathered record fields (f32): lx,ly,lz + 16 bf16 -> 64B
RECS = 64        # record row stride fields (f32): 256B rows (DGE stride quantum)
T = 4            # queries per chunk
J = NQ // T      # chunks per core = 512
JS = 64          # chunks per gather block == compute subtile
NSUB = J // JS   # 8
RADIUS = 0.1
DIL = 0.05
STAGE = int(os.environ.get("CONVSP_STAGE", "5"))
EJ = os.environ.get("CONVSP_EJ", "1") == "1"


def raw_dma_gather(eng, out_ap, in_ap, idxs_ap, num_idxs, elem_size, elem_step,
                   queue_num=0, single_packet=False):
    """dma_gather (transpose=False, DRAM source) without the %256
    elem-size assert — that restriction is only needed for transpose
    mode's 16-bit partition shuffle. elem_step (row stride) must still
    be a multiple of 256B, which is the DGE stride quantum."""
    eng._assert_queue_num(queue_num)
    assert idxs_ap.dtype == mybir.dt.int16
    assert in_ap.space == bass.MemorySpace.DRAM
    assert idxs_ap.space == bass.MemorySpace.SBUF
    assert out_ap.space == bass.MemorySpace.SBUF
    assert in_ap.dtype == out_ap.dtype
    assert ap_utils.ap_is_contiguous(in_ap.ap[1:])
    assert ap_utils.ap_is_contiguous(out_ap.ap[1:])
    assert ap_utils.ap_is_contiguous(idxs_ap.ap[1:])
    assert in_ap.ap[-1][1] == out_ap.ap[-1][1] == elem_size
    assert out_ap.ap[0][1] * out_ap.ap[1][1] == num_idxs  # num_idxs % 128 == 0
    assert in_ap.ap[0][0] == elem_step
    stride_bytes = elem_step * mybir.dt.size(in_ap.dtype)
    assert stride_bytes % 256 == 0
    stride_bytes_256 = stride_bytes // 256
    assert stride_bytes_256 < 256

    _in_ap = eng.lower_ap_dma(in_ap, for_custom_bir_dma=True)
    inst = eng.add_instruction(
        mybir.InstDMAGatherAnt(
            name=eng.bass.get_next_instruction_name(),
            ins=[
                *_in_ap,
                eng.lower_ap(idxs_ap),
                eng.lower_val_access(eng.to_reg(num_idxs)),
            ],
            outs=[eng.lower_ap(out_ap)],
            transpose=False,
            num_idxs=num_idxs,
            elem_size=elem_size,
            stride_bytes_256=stride_bytes_256,
            gen_mode=0,
            single_packet=single_packet,
            queue_num=queue_num,
            sbuf_tokens_per_rank=0,
            sbuf_free_dim_per_rank=0,
            sbuf_free_dim_pad_per_rank=0,
            sbuf_byte_offset=0,
        )
    )
    return inst


def build_program():
    nc = bacc.Bacc("TRN2", target_bir_lowering=False, debug=False,
                   num_devices=8, num_swdge_queues=4)

    rec_d = nc.declare_dram_parameter("rec", [N, RECS], F32, isOutput=False)
    qT3_d = nc.declare_dram_parameter("qT3", [P, J * D], F32, isOutput=False)
    w16_d = nc.declare_dram_parameter("w16", [16, J * 8], I16, isOutput=False)
    wbd_d = nc.declare_dram_parameter("wbd", [64, E * 64], BF16, isOutput=False)
    bias4_d = nc.declare_dram_parameter("bias4", [64], F32, isOutput=False)
    out_d = nc.declare_dram_parameter("out", [NQ, O], F32, isOutput=True)

    with tile.TileContext(nc) as tc:
        with ExitStack() as ctx:
            _build(ctx, tc, rec_d, qT3_d, w16_d, wbd_d, bias4_d, out_d)
    nc.finalize()
    return nc


def _build(ctx, tc, rec_d, qT3_d, w16_d, wbd_d, bias4_d, out_d):
    nc = tc.nc

    consts = ctx.enter_context(tc.tile_pool(name="consts", bufs=1))
    gpool = ctx.enter_context(tc.tile_pool(name="gpool", bufs=1))
    dve = ctx.enter_context(tc.tile_pool(name="dve", bufs=3))
    kvp = ctx.enter_context(tc.tile_pool(name="kv", bufs=3))
    accs = ctx.enter_context(tc.tile_pool(name="accs", bufs=1))
    outs = ctx.enter_context(tc.tile_pool(name="outs", bufs=int(os.environ.get("CONVSP_OUTS", "3"))))
    accps = ctx.enter_context(tc.tile_pool(name="accps", bufs=int(os.environ.get("CONVSP_APS", "4")), space="PSUM"))
    outps = ctx.enter_context(tc.tile_pool(name="outps", bufs=2, space="PSUM"))
    trps = ctx.enter_context(tc.tile_pool(name="trps", bufs=2, space="PSUM"))

    # ---------------- constants ----------------
    oxc = consts.tile([P, D], F32)      # 2*off(e)
    ox2c = consts.tile([P, D], F32)     # off(e)^2
    for i in range(D):
        off = (i - 1) * DIL
        nc.vector.memset(oxc[:, i:i + 1], 2.0 * off)
        nc.vector.memset(ox2c[:, i:i + 1], off * off)
    epsb = consts.tile([P, 1], F32)
    nc.vector.memset(epsb[:], 1e-12)
    oneb = consts.tile([P, 1], F32)
    nc.vector.memset(oneb[:], 1.0)
    ident = consts.tile([64, 64], F32)
    make_identity(nc, ident[:])
    warm = consts.tile([P, 1], F32)
    nc.vector.memset(warm[:], 1.0)
    nc.scalar.activation(warm[:], warm[:], AF.Sqrt, bias=epsb[:])
    nc.scalar.activation(warm[:], warm[:], AF.Relu, bias=oneb[:],
                         scale=-1.0 / RADIUS)
    # w16/qT3 loaded in per-subtile chunks so gather 0 / subtile 0 start
    # as soon as their slice lands; wbd/bias only gate the first final group
    w16 = consts.tile([P, J * 8], I16)  # [128, 4096]
    WC = J * 8 // NSUB
    QC = J * D // NSUB
    w16v = w16[:].rearrange("p (s c) -> p s c", s=NSUB)
    qT3 = consts.tile([P, J * D], F32)
    qT3v = qT3[:].rearrange("p (s c) -> p s c", s=NSUB)

    def w16_src(s):
        return w16_d[:, s * WC:(s + 1) * WC].partition_broadcast(8)

    def load_w16_chunk(s):
        nc.sync.dma_start(w16v[:, s], w16_src(s))

    def load_qT3_chunk(s):
        nc.scalar.dma_start(qT3v[:, s], qT3_d[:, s * QC:(s + 1) * QC])

    load_w16_chunk(0)
    wbd = consts.tile([64, E * 64], BF16)
    bias4 = consts.tile([64, 1], F32)

    # ---------------- gather + slotted buffers ----------------
    NGB = 4
    NSB = 3
    gbufs = [gpool.tile([P, JS * RECF], F32, tag=f"g{i}", name=f"g{i}")
             for i in range(NGB)]
    sbufs = [gpool.tile([P, JS * 64], BF16, tag=f"s{i}", name=f"s{i}")
             for i in range(NSB)]

    def gather_block(s):
        sm = s % NSUB
        g = gbufs[s % NGB]
        gvr = g[:].rearrange("p (j f) -> p j f", f=RECF)
        NSPL = int(os.environ.get("CONVSP_NSPL", "2"))
        cpq = JS // NSPL
        icols = JS * 8 // NSPL
        for q in range(NSPL):
            raw_dma_gather(
                nc.gpsimd,
                out_ap=gvr[:, q * cpq:(q + 1) * cpq, :],
                in_ap=rec_d[:, 0:RECF],
                idxs_ap=w16[:, sm * (JS * 8) + q * icols:
                            sm * (JS * 8) + (q + 1) * icols],
                num_idxs=cpq * P,
                elem_size=RECF,
                elem_step=RECS,
                queue_num=q,
            )

    def sub_pre(s):
        sm = s % NSUB
        g = gbufs[s % NGB]
        sbd = sbufs[s % NSB]
        gv = g[:].rearrange("p (j f) -> p j f", f=RECF)

        # --- block-diag data: t=0,2 as raw byte moves on DMA rings now;
        # t=1,3 as DVE 4x bf16 copies issued AFTER the kv chain so they
        # don't head-of-line-block d3 in the in-order DVE queue
        gb = g[:].bitcast(BF16).rearrange("p (j f) -> p j f", f=RECF * 2)
        sv = sbd[:].rearrange("p (j s c) -> p j s c", s=T, c=C)

        def slot(t):
            return (sv[t * K:(t + 1) * K, :, t, :],
                    gb[t * K:(t + 1) * K, :, 2 * D:2 * D + C])

        SLD = os.environ.get("CONVSP_SLD", "02")
        for t, eng in ((0, nc.sync), (2, nc.scalar)):
            if str(t) in SLD:
                dst, src = slot(t)
                eng.dma_start(dst, src)

        if EJ:
            # --- (e,j)-major per-axis-squares path: every op's inner dim is
            # the packed j run, so the broadcast adds hit DVE 2x bf16 mode;
            # summands (d_a+v)^2 are individually small => bf16-safe
            qs = qT3v[:, sm].rearrange("p (d j) -> p d j", j=JS)
            d3b = dve.tile([P, D * JS], BF16, tag="d3b")
            d3bv = d3b[:].rearrange("p (d j) -> p d j", j=JS)
            nc.vector.tensor_sub(d3bv, qs,
                                 gv[:, :, 0:D].rearrange("p j d -> p d j"))
            # t[a, v, j] = d_a + off_v  (3 fused-scalar adds, 4x mode)
            tav = dve.tile([P, 9 * JS], BF16, tag="tav")
            tavv = tav[:].rearrange("p (a v j) -> p a v j", a=D, v=D)
            for vb in range(D):
                nc.vector.tensor_scalar_add(tavv[:, :, vb, :], d3bv,
                                            (vb - 1) * DIL)
            # w = t^2 in one 2x mul
            wav = dve.tile([P, 9 * JS], BF16, tag="wav")
            nc.vector.tensor_mul(wav[:], tav[:], tav[:])
            wv = wav[:].rearrange("p (a v j) -> p a v j", a=D, v=D)
            wx, wy, wz = wv[:, 0], wv[:, 1], wv[:, 2]
            # u2[vx, vy, j] ; d2[vx, vy, vz, j] — 2x adds, packed inner j
            u2 = dve.tile([P, 9 * JS], BF16, tag="u2e")
            u2v = u2[:].rearrange("p (x y j) -> p x y j", x=D, y=D)
            nc.vector.tensor_add(
                u2v,
                wx.unsqueeze(2).broadcast_to((P, D, D, JS)),
                wy.unsqueeze(1).broadcast_to((P, D, D, JS)))
            d2 = kvp.tile([P, JS * E], BF16, tag="d2")
            d2v = d2[:].rearrange("p (u z j) -> p u z j", u=9, z=D)
            nc.vector.tensor_add(
                d2v,
                u2[:].rearrange("p (u j) -> p u j", j=JS).unsqueeze(2)
                     .broadcast_to((P, 9, D, JS)),
                wz.unsqueeze(1).broadcast_to((P, 9, D, JS)))
            d2f = d2
            nc.scalar.activation(d2f[:], d2f[:], AF.Sqrt, bias=epsb[:])
            y = kvp.tile([P, JS * E], BF16, tag="y")
            nc.scalar.activation(y[:], d2f[:], AF.Relu, bias=oneb[:],
                                 scale=-1.0 / RADIUS)
            return dict(s=s, sbd=sbd, slot=slot, y=y)

        # --- delta = q - l
        d3 = dve.tile([P, JS * D], F32, tag="d3")
        q_v = qT3[:].rearrange("p (j d) -> p j d", d=D)
        nc.vector.tensor_sub(
            d3[:].rearrange("p (j d) -> p j d", d=D),
            q_v[:, sm * JS:(sm + 1) * JS, :],
            gv[:, :, 0:D])

        d3v = d3[:].rearrange("p (j d) -> p j d", d=D)
        dx, dy, dz = d3v[:, :, 0], d3v[:, :, 1], d3v[:, :, 2]

        # --- s2 = dx^2+dy^2+dz^2
        s2 = dve.tile([P, JS], F32, tag="s2")
        tmp = dve.tile([P, JS], F32, tag="tmp")
        nc.vector.tensor_mul(s2[:], dx, dx)
        nc.vector.tensor_mul(tmp[:], dy, dy)
        nc.vector.tensor_add(s2[:], s2[:], tmp[:])
        nc.vector.tensor_mul(tmp[:], dz, dz)
        nc.vector.tensor_add(s2[:], s2[:], tmp[:])

        # --- per-axis terms p*[j,e] = 2*off_e*d + off_e^2 (+ s2 on x),
        # one fused scalar op per cell offset (off_e is a literal)
        def axis_term(dcomp, add_s2, tg):
            pt = dve.tile([P, JS * D], F32, tag=tg)
            ptv = pt[:].rearrange("p (j e) -> p j e", e=D)
            for e in range(D):
                off = (e - 1) * DIL
                if add_s2:
                    nc.vector.affine_then_add(ptv[:, :, e], dcomp, s2[:],
                                              scale=2.0 * off, bias=off * off)
                else:
                    nc.vector.tensor_scalar(ptv[:, :, e], dcomp,
                                            2.0 * off, off * off,
                                            mybir.AluOpType.mult,
                                            mybir.AluOpType.add)
            return pt

        pxe = axis_term(dx, True, "pxe")
        pye = axis_term(dy, False, "pye")
        pze = axis_term(dz, False, "pze")

        # --- u2[j,ex,ey] = pxe+pye ; d2[j,ex,ey,ez] = u2+pze
        u2 = dve.tile([P, JS * 9], F32, tag="u2")
        u2v = u2[:].rearrange("p (j a b) -> p j a b", a=D, b=D)
        nc.vector.tensor_add(
            u2v,
            pxe[:].rearrange("p (j a) -> p j a", a=D).unsqueeze(3)
                  .broadcast_to((P, JS, D, D)),
            pye[:].rearrange("p (j b) -> p j b", b=D).unsqueeze(2)
                  .broadcast_to((P, JS, D, D)))
        d2 = kvp.tile([P, JS * E], F32, tag="d2")
        d2v = d2[:].rearrange("p (j a b) -> p j a b", a=9, b=D)
        u2b = u2[:].rearrange("p (j a) -> p j a", a=9).unsqueeze(3) \
                   .broadcast_to((P, JS, 9, D))
        pzb = pze[:].rearrange("p (j b) -> p j b", b=D).unsqueeze(2) \
                    .broadcast_to((P, JS, 9, D))
        H = int(os.environ.get("CONVSP_D2POOL", "0"))
        if H:
            # split the biggest DVE op: first H j-columns on Pool
            nc.gpsimd.tensor_add(d2v[:, 0:H], u2b[:, 0:H], pzb[:, 0:H])
            nc.vector.tensor_add(d2v[:, H:], u2b[:, H:], pzb[:, H:])
        else:
            nc.vector.tensor_add(d2v, u2b, pzb)

        # --- kv = relu(1 - sqrt(d2+eps)/R)^3; y bf16 so the two cube muls
        # run in DVE 2x packed-16-bit mode
        nc.scalar.activation(d2[:], d2[:], AF.Sqrt, bias=epsb[:])
        y = kvp.tile([P, JS * E], BF16, tag="y")
        nc.scalar.activation(y[:], d2[:], AF.Relu, bias=oneb[:],
                             scale=-1.0 / RADIUS)
        return dict(s=s, sbd=sbd, slot=slot, y=y)

    def sub_post(st):
        s, sbd, slot, y = st["s"], st["sbd"], st["slot"], st["y"]
        sq = kvp.tile([P, JS * E], BF16, tag="sq")
        nc.vector.tensor_mul(sq[:], y[:], y[:])
        kv = kvp.tile([P, JS * E], BF16, tag="kvt")
        nc.vector.tensor_mul(kv[:], sq[:], y[:])

        SLD = os.environ.get("CONVSP_SLD", "02")
        for t in range(4):
            if str(t) not in SLD:
                dst, src = slot(t)
                nc.vector.tensor_copy(dst, src)

        # --- acc[(t,c), e] per chunk on PE (bf16: single-pass PE matmul)
        if EJ:
            kvv = kv[:].rearrange("p (e j) -> p j e", j=JS)
        else:
            kvv = kv[:].rearrange("p (j e) -> p j e", e=E)
        acc_sb = acc2[s % 2]
        for grp in range(JS // 16):
            ap_ps = accps.tile([64, 16 * E], F32, tag="accps")
            for jl in range(16):
                jj = grp * 16 + jl
                nc.tensor.matmul(ap_ps[:, jl * E:(jl + 1) * E],
                                 sbd[:, jj * 64:(jj + 1) * 64],
                                 kvv[:, jj, :],
                                 start=True, stop=True)
            nc.scalar.activation(acc_sb[:, grp * 16 * E:(grp + 1) * 16 * E],
                                 ap_ps[:], AF.Copy)

        final_group(s)

    def final_group(fg):
        # --- out[(t,o), j] = sum_e Wbd_e @ acc_e, one subtile (64 cols)
        JT = JS
        op = outps.tile([64, JT], F32, tag="outps")
        accv = acc2[fg % 2].rearrange("p (jt e) -> p jt e", e=E)
        for e in range(E):
            nc.tensor.matmul(op[:], wbd[:, e * 64:(e + 1) * 64],
                             accv[:, :, e],
                             start=(e == 0), stop=(e == E - 1))
        osb = outs.tile([64, JT], F32, tag="osb")
        nc.scalar.activation(osb[:], op[:], AF.Identity, bias=bias4[:])

        # --- transpose to [j, (t,o)] and store contiguously
        out_v = out_d[:].rearrange("(s j t) o -> s j (t o)", s=NSUB, t=T)
        for q in range(JT // 64):
            trp = trps.tile([64, 64], F32, tag="trp")
            nc.tensor.transpose(trp[:], osb[:, q * 64:(q + 1) * 64], ident[:])
            trs = outs.tile([64, 64], F32, tag="trs")
            nc.scalar.activation(trs[:], trp[:], AF.Copy)
            nc.sync.dma_start(out_v[fg % NSUB], trs[:])

    accbig = accs.tile([64, 2 * JS * E], BF16, tag="accbig", name="accbig")
    acc2 = [accbig[:, i * JS * E:(i + 1) * JS * E] for i in range(2)]

    # ---------------- pipeline ----------------
    # slotted-buffer zeroing on DVE: it is idle during the first gather.
    # sub_post(g-1) is issued after sub_pre(g) so the DVE ops that depend
    # on ACT output never head-of-line-block the next subtile's DVE work.
    reps = int(os.environ.get("CONVSP_REPS", "1"))
    total = NSUB * reps
    for b in range(NSB):
        nc.vector.memset(sbufs[b][:], 0.0)
    for gg in range(int(os.environ.get("CONVSP_GA", "2"))):
        if 1 <= gg < NSUB:
            load_w16_chunk(gg)
        gather_block(gg)
    load_qT3_chunk(0)
    load_qT3_chunk(1)
    nc.scalar.dma_start(wbd[:], wbd_d[:])
    nc.sync.dma_start(bias4[:], bias4_d[:].rearrange("(p o) -> p o", o=1))
    pending = []
    for g in range(total):
        GA = int(os.environ.get("CONVSP_GA", "2"))
        if g + GA < total:
            if 2 <= g + GA < NSUB:
                load_w16_chunk(g + GA)
            gather_block(g + GA)
        if 2 <= g + 1 < NSUB:
            load_qT3_chunk(g + 1)
        st = sub_pre(g)
        pending.append(st)
        if len(pending) > int(os.environ.get("CONVSP_PD", "1")):
            sub_post(pending.pop(0))
    while pending:
        sub_post(pending.pop(0))


_PROGRAM = None


def _get_program():
    global _PROGRAM
    if _PROGRAM is None:
        _PROGRAM = build_program()
    return _PROGRAM


def kernel(qlocs, locs, data, neighbors, weight, bias):
    import ml_dtypes

    B, M = qlocs.shape[0], qlocs.shape[1]
    assert (B, M) == (2, 8192)
    ncores = 8

    # host-side constant/layout prep: block-diagonal weights + replicated bias
    wbd = np.zeros((E, 64, 64), np.float32)
    w = np.asarray(weight, np.float32)           # [O, C, E]
    for t in range(T):
        # wbd[e, (t,c), (t,o)] = w[o, c, e]
        wbd[:, t * C:(t + 1) * C, t * O:(t + 1) * O] = w.transpose(2, 1, 0)
    wbd = np.ascontiguousarray(
        wbd.transpose(1, 0, 2).reshape(64, E * 64)).astype(ml_dtypes.bfloat16)
    bias4 = np.tile(np.asarray(bias, np.float32), T)

    # packed particle records: [lx,ly,lz f32 | data bf16 x16 | pad] per 256B row
    recs = []
    for b in range(B):
        rec = np.zeros((N, RECS), np.float32)
        rec[:, 0:D] = locs[b]
        rec.view(np.uint16)[:, 2 * D:2 * D + C] = (
            np.asarray(data[b], np.float32).astype(ml_dtypes.bfloat16)
            .view(np.uint16))
        recs.append(rec)

    in_maps = []
    for cid in range(ncores):
        b, qq = cid // 4, cid % 4
        sl = slice(qq * NQ, (qq + 1) * NQ)
        q = np.asarray(qlocs[b, sl], np.float32)          # [NQ, D]
        # qT3[(t*32+k), (j,d)] = q[4j+t, d]
        if EJ:
            # chunk-local (d, j)-major: qT3[p, (s, d, j64)] = q[4(s*64+j)+t, d]
            qsd = q.reshape(NSUB, JS, T, D).transpose(2, 0, 3, 1)  # [T,s,d,j]
            qT3 = np.ascontiguousarray(
                np.broadcast_to(qsd[:, None, :, :, :],
                                (T, K, NSUB, D, JS)).reshape(P, J * D))
        else:
            qT3 = np.ascontiguousarray(
                np.broadcast_to(
                    q.reshape(J, T, D).transpose(1, 0, 2)[:, None, :, :],
                    (T, K, J, D)).reshape(P, J * D))
        # wrap-format gather indices: flat stream i = m*32+k; w16[r, c] = flat[c*16+r]
        flat = np.asarray(neighbors[b, sl], np.int64).reshape(-1)
        w16 = np.ascontiguousarray(
            flat.reshape(J * 8, 16).T.astype(np.int16))
        in_maps.append({
            "rec": recs[b],
            "qT3": qT3,
            "w16": w16,
            "wbd": wbd,
            "bias4": bias4,
        })

    nc = _get_program()
    res = run_bass_kernel_spmd(nc, in_maps, list(range(ncores)),
                               trace=bool(int(os.environ.get("CONVSP_TRACE", "0"))))
    out = np.zeros((B, M, O), np.float32)
    for cid in range(ncores):
        b, qq = cid // 4, cid % 4
        out[b, qq * NQ:(qq + 1) * NQ] = res.results[cid]["out"]
    kernel.last_results = res
    return out
